# revision 15
# baseline (speedup 1.0000x reference)
"""LoRA QKV kernel for TRN2, 8 NeuronCores, data-parallel over rows.

y = x @ W_qkv^T + b_qkv ; q += (x a_q^T) b_q^T /16 ; v += (x a_v^T) b_v^T /16

Strategy:
 - shard the 4*4096=16384 rows across 8 cores (2048 rows each), replicate weights
 - host-side: transpose x shard to [K=1024, M=2048] and split all matmul operands
   into bf16 hi/lo pairs; f32 product reconstructed as xh@wh + xh@wl + xl@wh
   (error ~2^-18, PE runs at full bf16 rate)
 - LoRA is folded into the weights on the host: W' = W + scaling*B@A is a
   rank-16 update, exact algebraically, and dW ~ 2^-7.6 of W so the hi/lo
   split of W' captures it fully. The device runs a pure GEMM + bias.
 - bias added during the PSUM->SBUF copy (DVE tensor_add with host-replicated bias)
 - DMA order: small tensors + first x chunk first, then weights interleaved
   per 512-col n-tile so the first matmul group starts as early as possible
"""
import numpy as np
import ml_dtypes

import concourse.bass as bass
import concourse.mybir as mybir
import concourse.tile as tile
from concourse import bass_utils

D = 1024          # d_model (K)
NO = 3072         # 3 * nh_kd (N)
R = 16            # LoRA rank
SCALING = 1.0 / 16.0
N_CORES = 8
ROWS = 4 * 4096
M_CORE = ROWS // N_CORES      # 2048
KT = D // 128                 # 8 k-tiles
M_CHUNK = 512                 # rows per x-load chunk
N_TILE = 512                  # psum free dim
BF16 = ml_dtypes.bfloat16

TRACE = False
_CACHE = {}


def _split(a):
    hi = a.astype(BF16)
    lo = (a - hi.astype(np.float32)).astype(BF16)
    return np.ascontiguousarray(hi), np.ascontiguousarray(lo)


def _build_nc():
    nc = bass.Bass()
    dt = mybir.dt
    n_chunks_ = M_CORE // M_CHUNK
    n_tiles_ = NO // N_TILE
    xh_d = nc.dram_tensor("xh", (n_chunks_, 128, KT, M_CHUNK), dt.bfloat16, kind="ExternalInput")
    xl_d = nc.dram_tensor("xl", (n_chunks_, 128, KT, M_CHUNK), dt.bfloat16, kind="ExternalInput")
    wh_d = nc.dram_tensor("wh", (n_tiles_, 128, KT, N_TILE), dt.bfloat16, kind="ExternalInput")
    wl_d = nc.dram_tensor("wl", (n_tiles_, 128, KT, N_TILE), dt.bfloat16, kind="ExternalInput")
    bias_d = nc.dram_tensor("bias", (128, NO), dt.float32, kind="ExternalInput")
    out_d = nc.dram_tensor("out", (M_CORE, NO), dt.bfloat16, kind="ExternalOutput")

    n_chunks = M_CORE // M_CHUNK
    msubs = M_CHUNK // 128
    n_tiles = NO // N_TILE

    with tile.TileContext(nc) as tc:
        with tc.tile_pool(name="wres", bufs=1) as wres, \
             tc.tile_pool(name="xin", bufs=2) as xin, \
             tc.tile_pool(name="obuf", bufs=8) as obuf, \
             tc.tile_pool(name="psm", bufs=6, space="PSUM") as psm:

            def load_x_half(ch, which):
                t = xin.tile([128, KT, M_CHUNK], dt.bfloat16, tag=which)
                src = (xh_d if which == "xh" else xl_d)[ch]
                # split over kt pairs so the transfer round-robins across
                # multiple DGE engines (one dma_start = one engine)
                for k0 in range(0, KT, 2):
                    nc.sync.dma_start(t[:, k0:k0 + 2], src[:, k0:k0 + 2])
                return t

            def load_x(ch):
                return load_x_half(ch, "xh"), load_x_half(ch, "xl")

            wh_sb = wres.tile([128, n_tiles, KT, N_TILE], dt.bfloat16, tag="wh")
            wl_sb = wres.tile([128, n_tiles, KT, N_TILE], dt.bfloat16, tag="wl")

            def load_w(n):
                for k0 in range(0, KT, 2):
                    nc.sync.dma_start(wh_sb[:, n, k0:k0 + 2], wh_d[n, :, k0:k0 + 2])
                for k0 in range(0, KT, 2):
                    nc.sync.dma_start(wl_sb[:, n, k0:k0 + 2], wl_d[n, :, k0:k0 + 2])

            # startup critical path: xh -> wh0/wl0 -> xl unblocks the first
            # 24-matmul group term-by-term; bias arrives before first copy-out
            xh0 = load_x_half(0, "xh")
            load_w(0)
            xl0 = load_x_half(0, "xl")
            x_pending = (xh0, xl0)
            bias_sb = wres.tile([128, NO], dt.float32, tag="bias")
            nc.sync.dma_start(bias_sb[:], bias_d[:, :])
            scr = wres.tile([1, 16], dt.float32, tag="scr")
            # absorb the bias-DMA wait once so later tensor_adds carry none
            nc.vector.tensor_copy(scr[0:1, 0:1], bias_sb[0:1, 0:1])
            for n in range(1, n_tiles):
                load_w(n)

            for ch in range(n_chunks):
                m0 = ch * M_CHUNK
                xh_sb, xl_sb = x_pending
                if ch + 1 < n_chunks:
                    x_pending = load_x(ch + 1)

                for ms in range(msubs):
                    mm0 = ms * 128
                    for n in range(n_tiles):
                        nn0 = n * N_TILE
                        pm = psm.tile([128, N_TILE], dt.float32, tag="pm")
                        wterms = ((xh_sb, wh_sb), (xh_sb, wl_sb), (xl_sb, wh_sb))
                        for ti, (xx, ww) in enumerate(wterms):
                            for k in range(KT):
                                nc.tensor.matmul(
                                    pm[:, :],
                                    xx[:, k, mm0:mm0 + 128],
                                    ww[:, n, k, :],
                                    start=(ti == 0 and k == 0),
                                    stop=(ti == 2 and k == KT - 1))
                        ob = obuf.tile([128, N_TILE], dt.bfloat16, tag="ob")
                        # wait-absorbers: WAR on ob slot, RAW on pm (1 wait each)
                        nc.vector.memset(ob[0:1, 0:1], 0.0)
                        nc.vector.tensor_copy(scr[0:1, 1:2], pm[0:1, 0:1])
                        nc.vector.tensor_add(ob[:, :], pm[:, :], bias_sb[:, nn0:nn0 + N_TILE])
                        nc.sync.dma_start(
                            out_d[m0 + mm0:m0 + mm0 + 128, nn0:nn0 + N_TILE], ob[:, :])
    _split_multi_waits(nc)
    return nc


def _split_multi_waits(nc):
    """This walrus build fuses at most one sync-wait per instruction; hoist
    extras onto engine-matched NoOps inserted immediately before."""
    dt = mybir.dt
    uid = [0]
    for fn in nc.m.functions:
        for blk in fn.blocks:
            out = []
            for ins in blk.instructions:
                si = ins.sync_info
                waits = list(si.on_wait) if si is not None and si.on_wait else []
                if len(waits) > 1:
                    for w in waits[:-1]:
                        nop = mybir.InstNoOp(name=f"waitnop_{uid[0]}", ins=[], outs=[])
                        uid[0] += 1
                        nop.engine = ins.engine
                        nop.sync_info = mybir.SyncInfo(on_wait=[w], on_update=[])
                        out.append(nop)
                    ins.sync_info = mybir.SyncInfo(
                        on_wait=[waits[-1]],
                        on_update=list(si.on_update) if si.on_update else [])
                out.append(ins)
            blk.instructions = out


def _prep_shared(w_qkv, b_qkv, a_q, b_q, a_v, b_v):
    # fold the rank-16 LoRA update into W: W'^T = W^T + s*(A^T @ B^T)
    wT = np.ascontiguousarray(w_qkv.T.astype(np.float64))       # (1024, 3072)
    wT[:, 0:D] += SCALING * (a_q.T.astype(np.float64) @ b_q.T.astype(np.float64))
    wT[:, 2 * D:3 * D] += SCALING * (a_v.T.astype(np.float64) @ b_v.T.astype(np.float64))
    wh, wl = _split(wT.astype(np.float32))
    # tile [1024, 3072] -> [n_tiles, 128, KT, N_TILE] for contiguous DMA
    def tile_w(w):
        return np.ascontiguousarray(
            w.reshape(KT, 128, NO // N_TILE, N_TILE).transpose(2, 1, 0, 3))
    bias = np.ascontiguousarray(
        np.broadcast_to(b_qkv.astype(np.float32), (128, NO)))
    return tile_w(wh), tile_w(wl), bias


def kernel(x, w_qkv, b_qkv, a_q, b_q, a_v, b_v):
    x = np.asarray(x, np.float32)
    wh, wl, bias = _prep_shared(
        np.asarray(w_qkv), np.asarray(b_qkv), np.asarray(a_q),
        np.asarray(b_q), np.asarray(a_v), np.asarray(b_v))
    X = x.reshape(ROWS, D)
    in_maps = []
    for c in range(N_CORES):
        xT = np.ascontiguousarray(X[c * M_CORE:(c + 1) * M_CORE].T)
        xh, xl = _split(xT)
        # [1024, 2048] -> [n_chunks, 128, KT, M_CHUNK]
        xh = np.ascontiguousarray(
            xh.reshape(KT, 128, M_CORE // M_CHUNK, M_CHUNK).transpose(2, 1, 0, 3))
        xl = np.ascontiguousarray(
            xl.reshape(KT, 128, M_CORE // M_CHUNK, M_CHUNK).transpose(2, 1, 0, 3))
        in_maps.append({"xh": xh, "xl": xl, "wh": wh, "wl": wl, "bias": bias})
    if "nc" not in _CACHE:
        _CACHE["nc"] = _build_nc()
    nc = _CACHE["nc"]
    res = bass_utils.run_bass_kernel_spmd(
        nc, in_maps, core_ids=list(range(N_CORES)), trace=TRACE)
    if TRACE:
        _CACHE["last_exec_time_ns"] = res.exec_time_ns
        _CACHE["last_result"] = res
    out = np.concatenate([res.results[c]["out"].astype(np.float32) for c in range(N_CORES)], axis=0)
    return out.reshape(4, 4096, NO)


# revision 16
# speedup vs baseline: 1.0071x; 1.0071x over previous
"""LoRA QKV kernel for TRN2, 8 NeuronCores, data-parallel over rows.

y = x @ W_qkv^T + b_qkv ; q += (x a_q^T) b_q^T /16 ; v += (x a_v^T) b_v^T /16

Strategy:
 - shard the 4*4096=16384 rows across 8 cores (2048 rows each), replicate weights
 - host-side: transpose x shard to [K=1024, M=2048] and split all matmul operands
   into bf16 hi/lo pairs; f32 product reconstructed as xh@wh + xh@wl + xl@wh
   (error ~2^-18, PE runs at full bf16 rate)
 - LoRA is folded into the weights on the host: W' = W + scaling*B@A is a
   rank-16 update, exact algebraically, and dW ~ 2^-7.6 of W so the hi/lo
   split of W' captures it fully. The device runs a pure GEMM + bias.
 - bias added during the PSUM->SBUF copy (DVE tensor_add with host-replicated bias)
 - DMA order: small tensors + first x chunk first, then weights interleaved
   per 512-col n-tile so the first matmul group starts as early as possible
"""
import numpy as np
import ml_dtypes

import concourse.bass as bass
import concourse.mybir as mybir
import concourse.tile as tile
from concourse import bass_utils

D = 1024          # d_model (K)
NO = 3072         # 3 * nh_kd (N)
R = 16            # LoRA rank
SCALING = 1.0 / 16.0
N_CORES = 8
ROWS = 4 * 4096
M_CORE = ROWS // N_CORES      # 2048
KT = D // 128                 # 8 k-tiles
M_CHUNK = 512                 # rows per x-load chunk
N_TILE = 512                  # psum free dim
BF16 = ml_dtypes.bfloat16

TRACE = False
_CACHE = {}


def _split(a):
    hi = a.astype(BF16)
    lo = (a - hi.astype(np.float32)).astype(BF16)
    return np.ascontiguousarray(hi), np.ascontiguousarray(lo)


def _build_nc():
    nc = bass.Bass()
    dt = mybir.dt
    n_chunks_ = M_CORE // M_CHUNK
    n_tiles_ = NO // N_TILE
    xh_d = nc.dram_tensor("xh", (n_chunks_, 128, KT, M_CHUNK), dt.bfloat16, kind="ExternalInput")
    xl_d = nc.dram_tensor("xl", (n_chunks_, 128, KT, M_CHUNK), dt.bfloat16, kind="ExternalInput")
    wh_d = nc.dram_tensor("wh", (n_tiles_, 128, KT, N_TILE), dt.bfloat16, kind="ExternalInput")
    wl_d = nc.dram_tensor("wl", (n_tiles_, 128, KT, N_TILE), dt.bfloat16, kind="ExternalInput")
    bias_d = nc.dram_tensor("bias", (128, NO), dt.float32, kind="ExternalInput")
    out_d = nc.dram_tensor("out", (M_CORE, NO), dt.bfloat16, kind="ExternalOutput")

    n_chunks = M_CORE // M_CHUNK
    msubs = M_CHUNK // 128
    n_tiles = NO // N_TILE

    with tile.TileContext(nc) as tc:
        with tc.tile_pool(name="wres", bufs=1) as wres, \
             tc.tile_pool(name="xin", bufs=3) as xin, \
             tc.tile_pool(name="obuf", bufs=8) as obuf, \
             tc.tile_pool(name="psm", bufs=8, space="PSUM") as psm:

            def load_x_half(ch, which):
                t = xin.tile([128, KT, M_CHUNK], dt.bfloat16, tag=which)
                src = (xh_d if which == "xh" else xl_d)[ch]
                # split over kt pairs so the transfer round-robins across
                # multiple DGE engines (one dma_start = one engine)
                for k0 in range(0, KT, 2):
                    nc.sync.dma_start(t[:, k0:k0 + 2], src[:, k0:k0 + 2])
                return t

            def load_x(ch):
                return load_x_half(ch, "xh"), load_x_half(ch, "xl")

            wh_sb = wres.tile([128, n_tiles, KT, N_TILE], dt.bfloat16, tag="wh")
            wl_sb = wres.tile([128, n_tiles, KT, N_TILE], dt.bfloat16, tag="wl")

            def load_w(n):
                for k0 in range(0, KT, 2):
                    nc.sync.dma_start(wh_sb[:, n, k0:k0 + 2], wh_d[n, :, k0:k0 + 2])
                for k0 in range(0, KT, 2):
                    nc.sync.dma_start(wl_sb[:, n, k0:k0 + 2], wl_d[n, :, k0:k0 + 2])

            # startup critical path: xh -> wh0/wl0 -> xl unblocks the first
            # 24-matmul group term-by-term; bias arrives before first copy-out
            xh0 = load_x_half(0, "xh")
            load_w(0)
            xl0 = load_x_half(0, "xl")
            x_pending = (xh0, xl0)
            bias_sb = wres.tile([128, NO], dt.float32, tag="bias")
            nc.sync.dma_start(bias_sb[:], bias_d[:, :])
            scr = wres.tile([1, 16], dt.float32, tag="scr")
            # absorb the bias-DMA wait once so later tensor_adds carry none
            nc.vector.tensor_copy(scr[0:1, 0:1], bias_sb[0:1, 0:1])
            for n in range(1, n_tiles):
                load_w(n)

            for ch in range(n_chunks):
                m0 = ch * M_CHUNK
                xh_sb, xl_sb = x_pending
                if ch + 1 < n_chunks:
                    x_pending = load_x(ch + 1)

                for ms in range(msubs):
                    mm0 = ms * 128
                    for n in range(n_tiles):
                        nn0 = n * N_TILE
                        pm = psm.tile([128, N_TILE], dt.float32, tag="pm")
                        wterms = ((xh_sb, wh_sb), (xh_sb, wl_sb), (xl_sb, wh_sb))
                        for ti, (xx, ww) in enumerate(wterms):
                            for k in range(KT):
                                nc.tensor.matmul(
                                    pm[:, :],
                                    xx[:, k, mm0:mm0 + 128],
                                    ww[:, n, k, :],
                                    start=(ti == 0 and k == 0),
                                    stop=(ti == 2 and k == KT - 1))
                        ob = obuf.tile([128, N_TILE], dt.bfloat16, tag="ob")
                        # wait-absorbers: WAR on ob slot, RAW on pm (1 wait each)
                        nc.vector.memset(ob[0:1, 0:1], 0.0)
                        nc.vector.tensor_copy(scr[0:1, 1:2], pm[0:1, 0:1])
                        nc.vector.tensor_add(ob[:, :], pm[:, :], bias_sb[:, nn0:nn0 + N_TILE])
                        nc.sync.dma_start(
                            out_d[m0 + mm0:m0 + mm0 + 128, nn0:nn0 + N_TILE], ob[:, :])
    _split_multi_waits(nc)
    return nc


def _split_multi_waits(nc):
    """This walrus build fuses at most one sync-wait per instruction; hoist
    extras onto engine-matched NoOps inserted immediately before."""
    dt = mybir.dt
    uid = [0]
    for fn in nc.m.functions:
        for blk in fn.blocks:
            out = []
            for ins in blk.instructions:
                si = ins.sync_info
                waits = list(si.on_wait) if si is not None and si.on_wait else []
                if len(waits) > 1:
                    for w in waits[:-1]:
                        nop = mybir.InstNoOp(name=f"waitnop_{uid[0]}", ins=[], outs=[])
                        uid[0] += 1
                        nop.engine = ins.engine
                        nop.sync_info = mybir.SyncInfo(on_wait=[w], on_update=[])
                        out.append(nop)
                    ins.sync_info = mybir.SyncInfo(
                        on_wait=[waits[-1]],
                        on_update=list(si.on_update) if si.on_update else [])
                out.append(ins)
            blk.instructions = out


def _prep_shared(w_qkv, b_qkv, a_q, b_q, a_v, b_v):
    # fold the rank-16 LoRA update into W: W'^T = W^T + s*(A^T @ B^T)
    wT = np.ascontiguousarray(w_qkv.T.astype(np.float64))       # (1024, 3072)
    wT[:, 0:D] += SCALING * (a_q.T.astype(np.float64) @ b_q.T.astype(np.float64))
    wT[:, 2 * D:3 * D] += SCALING * (a_v.T.astype(np.float64) @ b_v.T.astype(np.float64))
    wh, wl = _split(wT.astype(np.float32))
    # tile [1024, 3072] -> [n_tiles, 128, KT, N_TILE] for contiguous DMA
    def tile_w(w):
        return np.ascontiguousarray(
            w.reshape(KT, 128, NO // N_TILE, N_TILE).transpose(2, 1, 0, 3))
    bias = np.ascontiguousarray(
        np.broadcast_to(b_qkv.astype(np.float32), (128, NO)))
    return tile_w(wh), tile_w(wl), bias


def kernel(x, w_qkv, b_qkv, a_q, b_q, a_v, b_v):
    x = np.asarray(x, np.float32)
    wh, wl, bias = _prep_shared(
        np.asarray(w_qkv), np.asarray(b_qkv), np.asarray(a_q),
        np.asarray(b_q), np.asarray(a_v), np.asarray(b_v))
    X = x.reshape(ROWS, D)
    in_maps = []
    for c in range(N_CORES):
        xT = np.ascontiguousarray(X[c * M_CORE:(c + 1) * M_CORE].T)
        xh, xl = _split(xT)
        # [1024, 2048] -> [n_chunks, 128, KT, M_CHUNK]
        xh = np.ascontiguousarray(
            xh.reshape(KT, 128, M_CORE // M_CHUNK, M_CHUNK).transpose(2, 1, 0, 3))
        xl = np.ascontiguousarray(
            xl.reshape(KT, 128, M_CORE // M_CHUNK, M_CHUNK).transpose(2, 1, 0, 3))
        in_maps.append({"xh": xh, "xl": xl, "wh": wh, "wl": wl, "bias": bias})
    if "nc" not in _CACHE:
        _CACHE["nc"] = _build_nc()
    nc = _CACHE["nc"]
    res = bass_utils.run_bass_kernel_spmd(
        nc, in_maps, core_ids=list(range(N_CORES)), trace=TRACE)
    if TRACE:
        _CACHE["last_exec_time_ns"] = res.exec_time_ns
        _CACHE["last_result"] = res
    out = np.concatenate([res.results[c]["out"].astype(np.float32) for c in range(N_CORES)], axis=0)
    return out.reshape(4, 4096, NO)


# revision 17
# speedup vs baseline: 1.0220x; 1.0147x over previous
"""LoRA QKV kernel for TRN2, 8 NeuronCores, data-parallel over rows.

y = x @ W_qkv^T + b_qkv ; q += (x a_q^T) b_q^T /16 ; v += (x a_v^T) b_v^T /16

Strategy:
 - shard the 4*4096=16384 rows across 8 cores (2048 rows each), replicate weights
 - host-side: transpose x shard to [K=1024, M=2048] and split all matmul operands
   into bf16 hi/lo pairs; f32 product reconstructed as xh@wh + xh@wl + xl@wh
   (error ~2^-18, PE runs at full bf16 rate)
 - LoRA is folded into the weights on the host: W' = W + scaling*B@A is a
   rank-16 update, exact algebraically, and dW ~ 2^-7.6 of W so the hi/lo
   split of W' captures it fully. The device runs a pure GEMM + bias.
 - bias added during the PSUM->SBUF copy (DVE tensor_add with host-replicated bias)
 - DMA order: small tensors + first x chunk first, then weights interleaved
   per 512-col n-tile so the first matmul group starts as early as possible
"""
import numpy as np
import ml_dtypes

import concourse.bass as bass
import concourse.mybir as mybir
import concourse.tile as tile
from concourse import bass_utils

D = 1024          # d_model (K)
NO = 3072         # 3 * nh_kd (N)
R = 16            # LoRA rank
SCALING = 1.0 / 16.0
N_CORES = 8
ROWS = 4 * 4096
M_CORE = ROWS // N_CORES      # 2048
KT = D // 128                 # 8 k-tiles
M_CHUNK = 512                 # rows per x-load chunk
N_TILE = 512                  # psum free dim
BF16 = ml_dtypes.bfloat16

TRACE = False
_CACHE = {}


def _split(a):
    hi = a.astype(BF16)
    lo = (a - hi.astype(np.float32)).astype(BF16)
    return np.ascontiguousarray(hi), np.ascontiguousarray(lo)


def _build_nc():
    nc = bass.Bass()
    dt = mybir.dt
    n_chunks_ = M_CORE // M_CHUNK
    n_tiles_ = NO // N_TILE
    xh_d = nc.dram_tensor("xh", (n_chunks_, 128, KT, M_CHUNK), dt.bfloat16, kind="ExternalInput")
    xl_d = nc.dram_tensor("xl", (n_chunks_, 128, KT, M_CHUNK), dt.bfloat16, kind="ExternalInput")
    wh_d = nc.dram_tensor("wh", (n_tiles_, 128, KT, N_TILE), dt.bfloat16, kind="ExternalInput")
    wl_d = nc.dram_tensor("wl", (n_tiles_, 128, KT, N_TILE), dt.bfloat16, kind="ExternalInput")
    bias_d = nc.dram_tensor("bias", (128, NO), dt.float32, kind="ExternalInput")
    out_d = nc.dram_tensor("out", (M_CORE, NO), dt.bfloat16, kind="ExternalOutput")

    n_chunks = M_CORE // M_CHUNK
    msubs = M_CHUNK // 128
    n_tiles = NO // N_TILE

    with tile.TileContext(nc) as tc:
        with tc.tile_pool(name="wres", bufs=1) as wres, \
             tc.tile_pool(name="xin", bufs=3) as xin, \
             tc.tile_pool(name="obuf", bufs=8) as obuf, \
             tc.tile_pool(name="psm", bufs=8, space="PSUM") as psm:

            def load_x_half(ch, which):
                t = xin.tile([128, KT, M_CHUNK], dt.bfloat16, tag=which)
                src = (xh_d if which == "xh" else xl_d)[ch]
                # split over kt pairs so the transfer round-robins across
                # multiple DGE engines (one dma_start = one engine)
                for k0 in range(0, KT, 2):
                    nc.sync.dma_start(t[:, k0:k0 + 2], src[:, k0:k0 + 2])
                return t

            def load_x(ch):
                return load_x_half(ch, "xh"), load_x_half(ch, "xl")

            wh_sb = wres.tile([128, n_tiles, KT, N_TILE], dt.bfloat16, tag="wh")
            wl_sb = wres.tile([128, n_tiles, KT, N_TILE], dt.bfloat16, tag="wl")

            def load_w(n):
                for k0 in range(0, KT, 2):
                    nc.sync.dma_start(wh_sb[:, n, k0:k0 + 2], wh_d[n, :, k0:k0 + 2])
                for k0 in range(0, KT, 2):
                    nc.sync.dma_start(wl_sb[:, n, k0:k0 + 2], wl_d[n, :, k0:k0 + 2])

            # startup critical path: xh -> wh0/wl0 -> xl unblocks the first
            # 24-matmul group term-by-term; bias arrives before first copy-out
            xh0 = load_x_half(0, "xh")
            load_w(0)
            xl0 = load_x_half(0, "xl")
            x_pending = (xh0, xl0)
            bias_sb = wres.tile([128, NO], dt.float32, tag="bias")
            nc.sync.dma_start(bias_sb[:], bias_d[:, :])
            scr = wres.tile([1, 16], dt.float32, tag="scr")
            # absorb the bias-DMA wait once so later tensor_adds carry none
            nc.vector.tensor_copy(scr[0:1, 0:1], bias_sb[0:1, 0:1])
            for n in range(1, n_tiles):
                load_w(n)

            for ch in range(n_chunks):
                m0 = ch * M_CHUNK
                xh_sb, xl_sb = x_pending
                if ch + 1 < n_chunks:
                    x_pending = load_x(ch + 1)

                # n-major so chunk-0 compute on n-tile t covers the DMA
                # stream of n-tile t+1 (~21us of cover per 2.1MB n-tile)
                for n in range(n_tiles):
                    nn0 = n * N_TILE
                    for ms in range(msubs):
                        mm0 = ms * 128
                        pm = psm.tile([128, N_TILE], dt.float32, tag="pm")
                        wterms = ((xh_sb, wh_sb), (xh_sb, wl_sb), (xl_sb, wh_sb))
                        for ti, (xx, ww) in enumerate(wterms):
                            for k in range(KT):
                                nc.tensor.matmul(
                                    pm[:, :],
                                    xx[:, k, mm0:mm0 + 128],
                                    ww[:, n, k, :],
                                    start=(ti == 0 and k == 0),
                                    stop=(ti == 2 and k == KT - 1))
                        ob = obuf.tile([128, N_TILE], dt.bfloat16, tag="ob")
                        # wait-absorbers: WAR on ob slot, RAW on pm (1 wait each)
                        nc.vector.memset(ob[0:1, 0:1], 0.0)
                        nc.vector.tensor_copy(scr[0:1, 1:2], pm[0:1, 0:1])
                        nc.vector.tensor_add(ob[:, :], pm[:, :], bias_sb[:, nn0:nn0 + N_TILE])
                        nc.sync.dma_start(
                            out_d[m0 + mm0:m0 + mm0 + 128, nn0:nn0 + N_TILE], ob[:, :])
    _split_multi_waits(nc)
    return nc


def _split_multi_waits(nc):
    """This walrus build fuses at most one sync-wait per instruction; hoist
    extras onto engine-matched NoOps inserted immediately before."""
    dt = mybir.dt
    uid = [0]
    for fn in nc.m.functions:
        for blk in fn.blocks:
            out = []
            for ins in blk.instructions:
                si = ins.sync_info
                waits = list(si.on_wait) if si is not None and si.on_wait else []
                if len(waits) > 1:
                    for w in waits[:-1]:
                        nop = mybir.InstNoOp(name=f"waitnop_{uid[0]}", ins=[], outs=[])
                        uid[0] += 1
                        nop.engine = ins.engine
                        nop.sync_info = mybir.SyncInfo(on_wait=[w], on_update=[])
                        out.append(nop)
                    ins.sync_info = mybir.SyncInfo(
                        on_wait=[waits[-1]],
                        on_update=list(si.on_update) if si.on_update else [])
                out.append(ins)
            blk.instructions = out


def _prep_shared(w_qkv, b_qkv, a_q, b_q, a_v, b_v):
    # fold the rank-16 LoRA update into W: W'^T = W^T + s*(A^T @ B^T)
    wT = np.ascontiguousarray(w_qkv.T.astype(np.float64))       # (1024, 3072)
    wT[:, 0:D] += SCALING * (a_q.T.astype(np.float64) @ b_q.T.astype(np.float64))
    wT[:, 2 * D:3 * D] += SCALING * (a_v.T.astype(np.float64) @ b_v.T.astype(np.float64))
    wh, wl = _split(wT.astype(np.float32))
    # tile [1024, 3072] -> [n_tiles, 128, KT, N_TILE] for contiguous DMA
    def tile_w(w):
        return np.ascontiguousarray(
            w.reshape(KT, 128, NO // N_TILE, N_TILE).transpose(2, 1, 0, 3))
    bias = np.ascontiguousarray(
        np.broadcast_to(b_qkv.astype(np.float32), (128, NO)))
    return tile_w(wh), tile_w(wl), bias


def kernel(x, w_qkv, b_qkv, a_q, b_q, a_v, b_v):
    x = np.asarray(x, np.float32)
    wh, wl, bias = _prep_shared(
        np.asarray(w_qkv), np.asarray(b_qkv), np.asarray(a_q),
        np.asarray(b_q), np.asarray(a_v), np.asarray(b_v))
    X = x.reshape(ROWS, D)
    in_maps = []
    for c in range(N_CORES):
        xT = np.ascontiguousarray(X[c * M_CORE:(c + 1) * M_CORE].T)
        xh, xl = _split(xT)
        # [1024, 2048] -> [n_chunks, 128, KT, M_CHUNK]
        xh = np.ascontiguousarray(
            xh.reshape(KT, 128, M_CORE // M_CHUNK, M_CHUNK).transpose(2, 1, 0, 3))
        xl = np.ascontiguousarray(
            xl.reshape(KT, 128, M_CORE // M_CHUNK, M_CHUNK).transpose(2, 1, 0, 3))
        in_maps.append({"xh": xh, "xl": xl, "wh": wh, "wl": wl, "bias": bias})
    if "nc" not in _CACHE:
        _CACHE["nc"] = _build_nc()
    nc = _CACHE["nc"]
    res = bass_utils.run_bass_kernel_spmd(
        nc, in_maps, core_ids=list(range(N_CORES)), trace=TRACE)
    if TRACE:
        _CACHE["last_exec_time_ns"] = res.exec_time_ns
        _CACHE["last_result"] = res
    out = np.concatenate([res.results[c]["out"].astype(np.float32) for c in range(N_CORES)], axis=0)
    return out.reshape(4, 4096, NO)


# revision 18
# speedup vs baseline: 1.0244x; 1.0024x over previous
"""LoRA QKV kernel for TRN2, 8 NeuronCores, data-parallel over rows.

y = x @ W_qkv^T + b_qkv ; q += (x a_q^T) b_q^T /16 ; v += (x a_v^T) b_v^T /16

Strategy:
 - shard the 4*4096=16384 rows across 8 cores (2048 rows each), replicate weights
 - host-side: transpose x shard to [K=1024, M=2048] and split all matmul operands
   into bf16 hi/lo pairs; f32 product reconstructed as xh@wh + xh@wl + xl@wh
   (error ~2^-18, PE runs at full bf16 rate)
 - LoRA is folded into the weights on the host: W' = W + scaling*B@A is a
   rank-16 update, exact algebraically, and dW ~ 2^-7.6 of W so the hi/lo
   split of W' captures it fully. The device runs a pure GEMM + bias.
 - bias added during the PSUM->SBUF copy (DVE tensor_add with host-replicated bias)
 - DMA order: small tensors + first x chunk first, then weights interleaved
   per 512-col n-tile so the first matmul group starts as early as possible
"""
import numpy as np
import ml_dtypes

import concourse.bass as bass
import concourse.mybir as mybir
import concourse.tile as tile
from concourse import bass_utils

D = 1024          # d_model (K)
NO = 3072         # 3 * nh_kd (N)
R = 16            # LoRA rank
SCALING = 1.0 / 16.0
N_CORES = 8
ROWS = 4 * 4096
M_CORE = ROWS // N_CORES      # 2048
KT = D // 128                 # 8 k-tiles
M_CHUNK = 512                 # rows per x-load chunk
N_TILE = 512                  # psum free dim
BF16 = ml_dtypes.bfloat16

TRACE = False
_CACHE = {}


def _split(a):
    hi = a.astype(BF16)
    lo = (a - hi.astype(np.float32)).astype(BF16)
    return np.ascontiguousarray(hi), np.ascontiguousarray(lo)


def _build_nc():
    nc = bass.Bass()
    dt = mybir.dt
    n_chunks_ = M_CORE // M_CHUNK
    n_tiles_ = NO // N_TILE
    xh_d = nc.dram_tensor("xh", (n_chunks_, 128, KT, M_CHUNK), dt.bfloat16, kind="ExternalInput")
    xl_d = nc.dram_tensor("xl", (n_chunks_, 128, KT, M_CHUNK), dt.bfloat16, kind="ExternalInput")
    wh_d = nc.dram_tensor("wh", (n_tiles_, 128, KT, N_TILE), dt.bfloat16, kind="ExternalInput")
    wl_d = nc.dram_tensor("wl", (n_tiles_, 128, KT, N_TILE), dt.bfloat16, kind="ExternalInput")
    bias_d = nc.dram_tensor("bias", (128, NO), dt.float32, kind="ExternalInput")
    out_d = nc.dram_tensor("out", (M_CORE, NO), dt.bfloat16, kind="ExternalOutput")

    n_chunks = M_CORE // M_CHUNK
    msubs = M_CHUNK // 128
    n_tiles = NO // N_TILE

    with tile.TileContext(nc) as tc:
        with tc.tile_pool(name="wres", bufs=1) as wres, \
             tc.tile_pool(name="xin", bufs=3) as xin, \
             tc.tile_pool(name="obuf", bufs=8) as obuf, \
             tc.tile_pool(name="psm", bufs=4, space="PSUM") as psm:

            def load_x_half(ch, which):
                t = xin.tile([128, KT, M_CHUNK], dt.bfloat16, tag=which)
                src = (xh_d if which == "xh" else xl_d)[ch]
                # split over kt pairs so the transfer round-robins across
                # multiple DGE engines (one dma_start = one engine)
                for k0 in range(0, KT, 2):
                    nc.sync.dma_start(t[:, k0:k0 + 2], src[:, k0:k0 + 2])
                return t

            def load_x(ch):
                return load_x_half(ch, "xh"), load_x_half(ch, "xl")

            wh_sb = wres.tile([128, n_tiles, KT, N_TILE], dt.bfloat16, tag="wh")
            wl_sb = wres.tile([128, n_tiles, KT, N_TILE], dt.bfloat16, tag="wl")

            def load_w(n):
                for k0 in range(0, KT, 2):
                    nc.sync.dma_start(wh_sb[:, n, k0:k0 + 2], wh_d[n, :, k0:k0 + 2])
                for k0 in range(0, KT, 2):
                    nc.sync.dma_start(wl_sb[:, n, k0:k0 + 2], wl_d[n, :, k0:k0 + 2])

            # startup critical path: xh -> wh0/wl0 -> xl unblocks the first
            # 24-matmul group term-by-term; bias arrives before first copy-out
            xh0 = load_x_half(0, "xh")
            load_w(0)
            xl0 = load_x_half(0, "xl")
            x_pending = (xh0, xl0)
            bias_sb = wres.tile([128, NO], dt.float32, tag="bias")
            nc.sync.dma_start(bias_sb[:], bias_d[:, :])
            scr = wres.tile([1, 16], dt.float32, tag="scr")
            # absorb the bias-DMA wait once so later tensor_adds carry none
            nc.vector.tensor_copy(scr[0:1, 0:1], bias_sb[0:1, 0:1])
            for n in range(1, n_tiles):
                load_w(n)

            for ch in range(n_chunks):
                m0 = ch * M_CHUNK
                xh_sb, xl_sb = x_pending
                if ch + 1 < n_chunks:
                    x_pending = load_x(ch + 1)

                # n-major so chunk-0 compute on n-tile t covers the DMA
                # stream of n-tile t+1 (~21us of cover per 2.1MB n-tile)
                for n in range(n_tiles):
                    nn0 = n * N_TILE
                    for ms in range(msubs):
                        mm0 = ms * 128
                        # 2-bank tile (only first N_TILE used): keeps each
                        # group's zero-region clear of its neighbor's bank
                        pm2 = psm.tile([128, 2 * N_TILE], dt.float32, tag="pm")
                        pm = pm2[:, 0:N_TILE]
                        wterms = ((xh_sb, wh_sb), (xh_sb, wl_sb), (xl_sb, wh_sb))
                        for ti, (xx, ww) in enumerate(wterms):
                            for k in range(KT):
                                nc.tensor.matmul(
                                    pm[:, :],
                                    xx[:, k, mm0:mm0 + 128],
                                    ww[:, n, k, :],
                                    start=(ti == 0 and k == 0),
                                    stop=(ti == 2 and k == KT - 1))
                        ob = obuf.tile([128, N_TILE], dt.bfloat16, tag="ob")
                        # wait-absorbers: WAR on ob slot, RAW on pm (1 wait each)
                        nc.vector.memset(ob[0:1, 0:1], 0.0)
                        nc.vector.tensor_copy(scr[0:1, 1:2], pm[0:1, 0:1])
                        nc.vector.tensor_add(ob[:, :], pm[:, :], bias_sb[:, nn0:nn0 + N_TILE])
                        nc.sync.dma_start(
                            out_d[m0 + mm0:m0 + mm0 + 128, nn0:nn0 + N_TILE], ob[:, :])
    _split_multi_waits(nc)
    return nc


def _split_multi_waits(nc):
    """This walrus build fuses at most one sync-wait per instruction; hoist
    extras onto engine-matched NoOps inserted immediately before."""
    dt = mybir.dt
    uid = [0]
    for fn in nc.m.functions:
        for blk in fn.blocks:
            out = []
            for ins in blk.instructions:
                si = ins.sync_info
                waits = list(si.on_wait) if si is not None and si.on_wait else []
                if len(waits) > 1:
                    for w in waits[:-1]:
                        nop = mybir.InstNoOp(name=f"waitnop_{uid[0]}", ins=[], outs=[])
                        uid[0] += 1
                        nop.engine = ins.engine
                        nop.sync_info = mybir.SyncInfo(on_wait=[w], on_update=[])
                        out.append(nop)
                    ins.sync_info = mybir.SyncInfo(
                        on_wait=[waits[-1]],
                        on_update=list(si.on_update) if si.on_update else [])
                out.append(ins)
            blk.instructions = out


def _prep_shared(w_qkv, b_qkv, a_q, b_q, a_v, b_v):
    # fold the rank-16 LoRA update into W: W'^T = W^T + s*(A^T @ B^T)
    wT = np.ascontiguousarray(w_qkv.T.astype(np.float64))       # (1024, 3072)
    wT[:, 0:D] += SCALING * (a_q.T.astype(np.float64) @ b_q.T.astype(np.float64))
    wT[:, 2 * D:3 * D] += SCALING * (a_v.T.astype(np.float64) @ b_v.T.astype(np.float64))
    wh, wl = _split(wT.astype(np.float32))
    # tile [1024, 3072] -> [n_tiles, 128, KT, N_TILE] for contiguous DMA
    def tile_w(w):
        return np.ascontiguousarray(
            w.reshape(KT, 128, NO // N_TILE, N_TILE).transpose(2, 1, 0, 3))
    bias = np.ascontiguousarray(
        np.broadcast_to(b_qkv.astype(np.float32), (128, NO)))
    return tile_w(wh), tile_w(wl), bias


def kernel(x, w_qkv, b_qkv, a_q, b_q, a_v, b_v):
    x = np.asarray(x, np.float32)
    wh, wl, bias = _prep_shared(
        np.asarray(w_qkv), np.asarray(b_qkv), np.asarray(a_q),
        np.asarray(b_q), np.asarray(a_v), np.asarray(b_v))
    X = x.reshape(ROWS, D)
    in_maps = []
    for c in range(N_CORES):
        xT = np.ascontiguousarray(X[c * M_CORE:(c + 1) * M_CORE].T)
        xh, xl = _split(xT)
        # [1024, 2048] -> [n_chunks, 128, KT, M_CHUNK]
        xh = np.ascontiguousarray(
            xh.reshape(KT, 128, M_CORE // M_CHUNK, M_CHUNK).transpose(2, 1, 0, 3))
        xl = np.ascontiguousarray(
            xl.reshape(KT, 128, M_CORE // M_CHUNK, M_CHUNK).transpose(2, 1, 0, 3))
        in_maps.append({"xh": xh, "xl": xl, "wh": wh, "wl": wl, "bias": bias})
    if "nc" not in _CACHE:
        _CACHE["nc"] = _build_nc()
    nc = _CACHE["nc"]
    res = bass_utils.run_bass_kernel_spmd(
        nc, in_maps, core_ids=list(range(N_CORES)), trace=TRACE)
    if TRACE:
        _CACHE["last_exec_time_ns"] = res.exec_time_ns
        _CACHE["last_result"] = res
    out = np.concatenate([res.results[c]["out"].astype(np.float32) for c in range(N_CORES)], axis=0)
    return out.reshape(4, 4096, NO)
